# revision 63
# baseline (speedup 1.0000x reference)
"""Trainium2 Bass kernel for ContinuousDGM message passing.

  xe = x @ W_emb + b_emb            [N, E]
  D  = sq_cdist(xe)                 [N, N]
  A  = 1 / (1 + D)
  W  = A / A.sum(axis=1)            (broadcast over last axis -> col-normalize)
  out = W @ xe                      [N, E]

v2.3 strategy (8 NeuronCores, row-block sharding, fully fused; [N,N] never
touches DRAM). Per core: a 1024-column block, all 8192 rows.

  * Pre-G: xe^T supers on PE (W stationary, fat x block DMAs over three
    queues); PE transposes build row-major xe_bf in 16-tile batches;
    squares batched (ACT Square + DVE grouped reduce, f32 junk so the
    row norms match the PE's f32 products exactly); sq rides as bf16
    hi/lo aug-row pairs so the diagonal of 1+D cancels to ~1e-3
    (out[i] ~ ye[i] + similar-sized random-sign noise, so the diagonal
    weight needs that accuracy).
  * A-pass per j-tile: 2 G matmuls -> psum [128,1024] f32 = 1+D, then
    the reciprocal splits across engines: ~2/3 of tiles on ACT via raw
    InstActivation Reciprocal (table is bf16-accurate; one pass does
    recip + bf16 cast + row-sum accum_out), ~1/3 on DVE via
    reciprocal_approx_fast + tensor_scalar(mult 1.0, op1=add accum).
  * Row-sum partials all-reduced in 3 staggered chunks (gpsimd
    AllReduce; the CC ring serializes collectives so few chunks beat
    many); ye = xe/s applied in place per chunk with one broadcast
    tensor_tensor.
  * Out pass: 128 accumulating matmuls into po [64,1024]; runs with
    ACT/DVE idle so HAM un-throttles the PE toward 2.4 GHz; the last
    chunk is small so the final AllReduce hides under earlier tiles.
  * Host concatenates the 8 outT blocks and transposes.
"""

import os
import sys

import numpy as np

N, DIN, E = 8192, 256, 64
P = 128
C = 8
B = N // C            # 1024 cols per core
SUP = 512
NSUP = N // SUP       # 16
BSUP = B // SUP       # 2
NT = N // P           # 64
BT = B // P           # 8
AUG = 68              # 64 xe rows + 2 ones + sq hi/lo
GRP = 16              # j-tiles per pre-G batch
NG = NT // GRP        # 4 pre-G groups
CH = [(0, 12), (12, 30), (30, 48), (48, 64)]
NCH = len(CH)
DVE_LANE = [(jt * 23) % 64 < 23 for jt in range(NT)]   # 23 tiles on DVE

_NC_CACHE = {}
DEBUG_DUMPS = False


def _import_concourse():
    try:
        import concourse.bacc  # noqa: F401
    except ImportError:
        for p in ("/opt/trn_rl_repo", "/root/.axon_site/_ro/trn_rl_repo"):
            if os.path.isdir(p) and p not in sys.path:
                sys.path.insert(0, p)
        import concourse.bacc  # noqa: F401


def _act_raw(nc, out, in_, func, accum_out=None, scale=1.0, bias=0.0):
    """Emit InstActivation directly (the helper blocks Reciprocal; its
    table is accurate to ~4e-3 which is fine at our 2e-2 tolerance)."""
    from concourse import mybir

    eng = nc.scalar
    inputs = [eng.lower_ap(in_)]
    for arg in (bias, scale, 0.0):
        if isinstance(arg, float):
            inputs.append(mybir.ImmediateValue(dtype=mybir.dt.float32, value=arg))
        else:
            inputs.append(eng.lower_ap(arg))
    outputs = [eng.lower_ap(out)]
    if accum_out is not None:
        outputs.append(eng.lower_ap(accum_out))
    return eng.add_instruction(mybir.InstActivation(
        name=nc.get_next_instruction_name(),
        func=func, ins=inputs, outs=outputs))


def build_body(tc, outT, xT, xTl, W, b, eye, dbg_aps=None):
    from contextlib import ExitStack

    from concourse import mybir
    from concourse.bass import broadcast_tensor_aps

    if DEBUG_DUMPS:
        (dbg_sq, dbg_r, dbg_s, dbg_a0, dbg_a2, dbg_xe, dbg_augL,
         dbg_augR) = dbg_aps

    nc = tc.nc
    f32 = mybir.dt.float32
    bf16 = mybir.dt.bfloat16
    AF = mybir.ActivationFunctionType
    ALU = mybir.AluOpType

    with ExitStack() as ctx:
        big = ctx.enter_context(tc.tile_pool(name="big", bufs=1))
        const = ctx.enter_context(tc.tile_pool(name="const", bufs=1))
        work = ctx.enter_context(tc.tile_pool(name="work", bufs=1))
        dram = ctx.enter_context(tc.tile_pool(name="dram", bufs=1, space="DRAM"))

        # ---------- constants ----------
        Wsb = const.tile([P, 2, E], bf16, name="Wsb", tag="Wsb")
        for t in range(2):
            nc.sync.dma_start(Wsb[:, t, :], W[t * P:(t + 1) * P, :])
        b_col = const.tile([E, 1], f32, name="bcol", tag="bcol")
        nc.scalar.dma_start(b_col[:], b[:])
        b2_col = const.tile([E, 1], f32, name="b2col", tag="b2col")
        nc.vector.tensor_scalar_mul(b2_col[:], b_col[:], -2.0)
        eye_f = const.tile([P, P], f32, name="eyef", tag="eyef")
        nc.scalar.dma_start(eye_f[:], eye[:])
        eye_b = const.tile([P, P], bf16, name="eyeb", tag="eyeb")
        nc.scalar.copy(eye_b[:], eye_f[:])

        # ---------- operand buffers ----------
        # augL: [0:64]=-2*xeT, [64:66]=ones, [66]=sq_hi, [67]=sq_lo
        # augRl: [0:64]=xeT local, [64]=(1+sq)hi, [65]=(1+sq)lo, [66:68]=ones
        augL = big.tile([AUG, N], bf16, name="augL", tag="augL")
        augRl = big.tile([AUG, B], bf16, name="augRl", tag="augRl")
        # ones rows: engines can only address partition starts 0/32/64/96;
        # stage small memset tiles and DMA into rows >=64.
        ones_st = work.tile([64, 2 * P], bf16, name="ones_st", tag="ones_st")
        nc.vector.memset(ones_st[:], 1.0)
        nc.scalar.dma_start(augL[64:66, :], ones_st[:, :])
        nc.scalar.dma_start(augRl[66:68, :], ones_st[0:16, 0:P])

        xe_bf = big.tile([P, NT, E], bf16, name="xebf", tag="xebf")
        sq_mat = const.tile([P, NT], f32, name="sqmat", tag="sqmat")
        sql_mat = const.tile([P, BT], f32, name="sqlmat", tag="sqlmat")
        r_mat = const.tile([P, NT], f32, name="rmat", tag="rmat")
        junk = work.tile([P, GRP, E], f32, name="junk", tag="junk", bufs=2)
        srow_hi = work.tile([GRP, P], bf16, name="srow_hi", tag="srow_hi",
                            bufs=2)
        srow_lo = work.tile([GRP, P], bf16, name="srow_lo", tag="srow_lo",
                            bufs=2)
        hif = work.tile([GRP, P], f32, name="hif", tag="hif", bufs=2)
        lof = work.tile([GRP, P], f32, name="lof", tag="lof", bufs=2)

        atb = [big.tile([P, B], bf16, name=f"atb{jt}", tag=f"atb{jt}")
               for jt in range(NT)]

        agin, agout, s_sb, rs = [], [], [], []
        for h, (c0, c1) in enumerate(CH):
            w = c1 - c0
            agin.append(dram.tile([P * w], f32, name=f"agin{h}",
                                  tag=f"agin{h}"))
            agout.append(dram.tile([P * w], f32, name=f"agout{h}",
                                   tag=f"agout{h}", addr_space="Shared"))
            s_sb.append(const.tile([P, w, 1], f32, name=f"ssb{h}",
                                   tag=f"ssb{h}"))
            rs.append(const.tile([P, w, 1], f32, name=f"rs{h}", tag=f"rs{h}"))

        # ================= pre-G phase =================
        with tc.tile_pool(name="psum_pre", bufs=1, space="PSUM") as psum_pre:
            queues = [nc.sync, nc.gpsimd, nc.scalar]
            qctr = [0]

            def load_block(xsrc, cb, width, tag, bufs, nsub=1):
                tiles = []
                w = width // nsub
                for t in range(2):
                    xc = work.tile([P, width], bf16, name=tag, tag=tag,
                                   bufs=bufs)
                    for u in range(nsub):
                        q = queues[qctr[0] % len(queues)]
                        qctr[0] += 1
                        q.dma_start(
                            xc[:, u * w:(u + 1) * w],
                            xsrc[t * P:(t + 1) * P,
                                 cb * width + u * w:cb * width + (u + 1) * w])
                    tiles.append(xc)
                return tiles

            def emit_xeT_from(tiles, off, dst, s, lneg):
                ps = psum_pre.tile([E, SUP], f32, name="ps", tag="ps", bufs=3)
                for t in range(2):
                    nc.tensor.matmul(
                        ps[:], lhsT=Wsb[:, t, :],
                        rhs=tiles[t][:, off:off + SUP],
                        start=(t == 0), stop=(t == 1),
                    )
                sl = slice(s * SUP, (s + 1) * SUP)
                if lneg:
                    nc.scalar.activation(dst[0:64, sl], ps[:], AF.Identity,
                                         bias=b2_col[:], scale=-2.0)
                else:
                    nc.scalar.activation(dst[0:64, sl], ps[:], AF.Identity,
                                         bias=b_col[:], scale=1.0)

            def hi_lo_rows(psq, nt, plus_one, dst_hi, dst_lo):
                """psq[0:nt] (f32 sq values) -> bf16 hi/lo rows + DMA out."""
                if plus_one:
                    nc.scalar.activation(srow_hi[0:nt, :], psq[0:nt, :],
                                         AF.Identity, bias=1.0)
                else:
                    nc.scalar.copy(srow_hi[0:nt, :], psq[0:nt, :])
                nc.vector.tensor_copy(out=hif[0:nt, :], in_=srow_hi[0:nt, :])
                if plus_one:
                    nc.vector.tensor_scalar_add(lof[0:nt, :], psq[0:nt, :],
                                                1.0)
                    nc.vector.tensor_tensor(lof[0:nt, :], lof[0:nt, :],
                                            hif[0:nt, :], ALU.subtract)
                else:
                    nc.vector.tensor_tensor(lof[0:nt, :], psq[0:nt, :],
                                            hif[0:nt, :], ALU.subtract)
                nc.vector.tensor_copy(out=srow_lo[0:nt, :], in_=lof[0:nt, :])
                nc.sync.dma_start(dst_hi, srow_hi[0:nt, :])
                nc.sync.dma_start(dst_lo, srow_lo[0:nt, :])

            # ---- local block: augRl + sq_local ----
            xtl_tiles = load_block(xTl, 0, B, "xcl", 1, nsub=2)
            blocks = {0: load_block(xT, 0, 4 * SUP, "xcb", 4, nsub=2)}
            for s in range(BSUP):
                emit_xeT_from(xtl_tiles, s * SUP, augRl, s, False)
            ptg = psum_pre.tile([P, GRP, E], bf16, name="ptg", tag="ptg",
                                bufs=2)
            psq = psum_pre.tile([GRP, P], f32, name="psq", tag="psq", bufs=2)
            for it in range(BT):
                nc.tensor.transpose(ptg[:, it, :],
                                    augRl[0:64, it * P:(it + 1) * P],
                                    eye_b[0:64, 0:64])
            # squares of local xe: ptg holds xe (bf16) directly
            nc.scalar.activation(junk[:, 0:BT, :], ptg[:, 0:BT, :], AF.Square)
            nc.vector.tensor_reduce(out=sql_mat[:], in_=junk[:, 0:BT, :],
                                    axis=mybir.AxisListType.X, op=ALU.add)
            nc.tensor.transpose(psq[0:BT, :], sql_mat[:], eye_f[:])
            hi_lo_rows(psq, BT, True, augRl[64:65, :], augRl[65:66, :])

            # ---- full rows: augL, xe_bf, sq row, in 4 groups ----
            for g in range(NG):
                if g + 1 < NG:
                    blocks[g + 1] = load_block(xT, g + 1, 4 * SUP, "xcb", 4)
                ptg = psum_pre.tile([P, GRP, E], bf16, name="ptg", tag="ptg",
                                    bufs=2)
                psq = psum_pre.tile([GRP, P], f32, name="psq", tag="psq",
                                    bufs=2)
                for si in range(4):
                    s = g * 4 + si
                    emit_xeT_from(blocks[g], si * SUP, augL, s, True)
                    for ti in range(4):
                        it = s * 4 + ti
                        nc.tensor.transpose(
                            ptg[:, it - g * GRP, :],
                            augL[0:64, it * P:(it + 1) * P],
                            eye_b[0:64, 0:64])
                gsl = slice(g * GRP, (g + 1) * GRP)
                # xe_bf = -0.5 * (-2 xe)  (exact)
                nc.vector.tensor_scalar_mul(xe_bf[:, gsl, :], ptg[:, :, :],
                                            -0.5)
                nc.scalar.activation(junk[:, :, :], xe_bf[:, gsl, :],
                                     AF.Square)
                nc.vector.tensor_reduce(out=sq_mat[:, gsl], in_=junk[:, :, :],
                                        axis=mybir.AxisListType.X, op=ALU.add)
                nc.tensor.transpose(psq[:, :], sq_mat[:, gsl], eye_f[:])
                gp = slice(g * GRP * P, (g + 1) * GRP * P)
                hi_lo_rows(psq, GRP, False, augL[66:67, gp], augL[67:68, gp])

        # ================= A-pass + out =================
        with tc.tile_pool(name="psum_a", bufs=1, space="PSUM") as psum_a:
            po = psum_a.tile([E, B], f32, name="po", tag="po", bufs=1)

            def flush(h):
                c0, c1 = CH[h]
                nc.gpsimd.dma_start(agin[h][:], r_mat[:, c0:c1])
                nc.gpsimd.collective_compute(
                    "AllReduce", ALU.add,
                    replica_groups=[list(range(C))],
                    ins=[agin[h][:]], outs=[agout[h][:]],
                )
                nc.gpsimd.dma_start(s_sb[h][:, :, :], agout[h][:])

            def ye(h):
                c0, c1 = CH[h]
                nc.vector.reciprocal(rs[h][:, :, :], s_sb[h][:, :, :])
                b0, b1 = broadcast_tensor_aps(xe_bf[:, c0:c1, :],
                                              rs[h][:, :, 0:1])
                nc.vector.tensor_tensor(xe_bf[:, c0:c1, :], b0, b1, ALU.mult)

            ch_of_end = {c1 - 1: h for h, (c0, c1) in enumerate(CH)}
            for jt in range(NT):
                pg = psum_a.tile([P, B], f32, name="pg", tag="pg", bufs=3)
                for h in range(BSUP):
                    nc.tensor.matmul(pg[:, h * SUP:(h + 1) * SUP],
                                     lhsT=augL[:, jt * P:(jt + 1) * P],
                                     rhs=augRl[:, h * SUP:(h + 1) * SUP],
                                     start=True, stop=True)
                if DVE_LANE[jt]:
                    # DVE lane: recip, then cast + row-sum in one
                    # tensor_scalar (accum needs op1)
                    ar = work.tile([P, B], f32, name="ar", tag="ar", bufs=2)
                    nc.vector.reciprocal_approx_fast(out=ar[:], in_=pg[:])
                    nc.vector.tensor_scalar(
                        out=atb[jt][:], in0=ar[:], scalar1=1.0, scalar2=None,
                        op0=ALU.mult, op1=ALU.add,
                        accum_out=r_mat[:, jt:jt + 1])
                else:
                    # ACT lane: one pass does recip + bf16 cast + row-sum
                    _act_raw(nc, atb[jt][:], pg[:], AF.Reciprocal,
                             accum_out=r_mat[:, jt:jt + 1])
                if jt in ch_of_end:
                    h = ch_of_end[jt]
                    flush(h)
                    if h >= 1:
                        ye(h - 1)

            # ---- out pass (last chunk's ye waits on the final AllReduce;
            # the earlier tiles' matmuls hide that latency)
            def out_tile(k):
                for h in range(BSUP):
                    nc.tensor.matmul(po[:, h * SUP:(h + 1) * SUP],
                                     lhsT=xe_bf[:, k, :],
                                     rhs=atb[k][:, h * SUP:(h + 1) * SUP],
                                     start=(k == 0), stop=(k == NT - 1))

            for k in range(0, CH[-1][0]):
                out_tile(k)
            ye(NCH - 1)
            # keep the PE busy across the final AllReduce wait so HAM does
            # not re-throttle (idle >3.4us drops the clock): a few dead
            # matmuls into the now-dead pg ring bridge the gap
            for _ in range(4):
                pgd = psum_a.tile([P, B], f32, name="pg", tag="pg", bufs=3)
                nc.tensor.matmul(pgd[:, 0:SUP], lhsT=augL[:, 0:P],
                                 rhs=augRl[:, 0:SUP], start=True, stop=True)
            for k in range(CH[-1][0], NT):
                out_tile(k)

            osb = work.tile([E, B], f32, name="osb", tag="osb")
            nc.scalar.copy(osb[:, 0:SUP], po[:, 0:SUP])
            nc.vector.tensor_copy(out=osb[:, SUP:B], in_=po[:, SUP:B])
            nc.sync.dma_start(outT[:, 0:SUP], osb[:, 0:SUP])
            nc.scalar.dma_start(outT[:, SUP:B], osb[:, SUP:B])

            if DEBUG_DUMPS:
                dbg = tc.nc  # alias
                dbg.sync.dma_start(dbg_sq[:, :], sq_mat[:, :])
                dbg.sync.dma_start(dbg_r[:, :], r_mat[:, :])
                for h, (c0, c1) in enumerate(CH):
                    dbg.sync.dma_start(dbg_s[:, c0:c1], s_sb[h][:, :, 0])
                dbg.sync.dma_start(dbg_a0[:, :], atb[0][:])
                dbg.sync.dma_start(dbg_a2[:, :], atb[2][:])
                dbg.sync.dma_start(dbg_xe[:, :, :], xe_bf[:, :, :])
                dbg.sync.dma_start(dbg_augL[:, :], augL[:, :])
                dbg.sync.dma_start(dbg_augR[:, :], augRl[:, :])


def _build_nc():
    _import_concourse()
    import concourse.bacc as bacc
    import concourse.tile as tile
    from concourse import mybir

    f32 = mybir.dt.float32
    bf16 = mybir.dt.bfloat16
    nc = bacc.Bacc("TRN2", target_bir_lowering=False, debug=False,
                   num_devices=C)
    xT = nc.dram_tensor("xT", [DIN, N], bf16, kind="ExternalInput").ap()
    xTl = nc.dram_tensor("xTl", [DIN, B], bf16, kind="ExternalInput").ap()
    W = nc.dram_tensor("W", [DIN, E], bf16, kind="ExternalInput").ap()
    b = nc.dram_tensor("b", [E, 1], f32, kind="ExternalInput").ap()
    eye = nc.dram_tensor("eye", [P, P], f32, kind="ExternalInput").ap()
    outT = nc.dram_tensor("outT", [E, B], f32, kind="ExternalOutput").ap()

    dbg_aps = None
    if DEBUG_DUMPS:
        dbg_aps = (
            nc.dram_tensor("dbg_sq", [P, NT], f32, kind="ExternalOutput").ap(),
            nc.dram_tensor("dbg_r", [P, NT], f32, kind="ExternalOutput").ap(),
            nc.dram_tensor("dbg_s", [P, NT], f32, kind="ExternalOutput").ap(),
            nc.dram_tensor("dbg_a0", [P, B], bf16, kind="ExternalOutput").ap(),
            nc.dram_tensor("dbg_a2", [P, B], bf16, kind="ExternalOutput").ap(),
            nc.dram_tensor("dbg_xe", [P, NT, E], bf16,
                           kind="ExternalOutput").ap(),
            nc.dram_tensor("dbg_augL", [AUG, N], bf16,
                           kind="ExternalOutput").ap(),
            nc.dram_tensor("dbg_augR", [AUG, B], bf16,
                           kind="ExternalOutput").ap(),
        )

    with tile.TileContext(nc) as tc:
        build_body(tc, outT, xT, xTl, W, b, eye, dbg_aps)
    nc.compile()
    return nc


def make_in_maps(x, W_emb, b_emb):
    import ml_dtypes

    bf = ml_dtypes.bfloat16
    xT = np.ascontiguousarray(x.T).astype(bf)
    eye = np.eye(P, dtype=np.float32)
    bb = np.asarray(b_emb, dtype=np.float32).reshape(E, 1)
    Wf = np.asarray(W_emb, dtype=np.float32).astype(bf)
    in_maps = []
    for c in range(C):
        in_maps.append({
            "xT": xT,
            "xTl": np.ascontiguousarray(xT[:, c * B:(c + 1) * B]),
            "W": Wf,
            "b": bb,
            "eye": eye,
        })
    return in_maps


def kernel(x, W_emb, b_emb, _trace=False, _tmpdir=None):
    _import_concourse()
    from concourse import bass_utils

    key = "nc"
    if key not in _NC_CACHE:
        _NC_CACHE[key] = _build_nc()
    nc = _NC_CACHE[key]

    in_maps = make_in_maps(np.asarray(x), np.asarray(W_emb), np.asarray(b_emb))
    res = bass_utils.run_bass_kernel_spmd(
        nc, in_maps, core_ids=list(range(C)),
        trace=_trace, tmpdir=_tmpdir,
    )
    blocks = [np.asarray(res.results[c]["outT"]) for c in range(C)]
    outT = np.concatenate(blocks, axis=1)          # [E, N]
    out = np.ascontiguousarray(outT.T).astype(np.float32)  # [N, E]
    if _trace:
        return out, res
    return out
